# revision 1
# baseline (speedup 1.0000x reference)
"""GAT-style message passing (nn_MicroConv) on 8 Trainium2 NeuronCores.

Strategy (dst-node partition, per the sharding hint's second option):
  - Each core owns N_DST/8 destination nodes and all edges into them.
  - Host: sort edges by dst, bucket into 32-dst windows, assign windows to
    fixed-size slot ranks (sorted by size so every core shares one compiled
    schedule), pad edge tiles of 128 to the slot grid with sentinel edges.
  - Device phase 0: per-core replicated node transform producing a gather
    table [N_src+1, 132] = [fs (128 cols) | e_src (4 cols)] plus a local
    e_dst table [n_dst_core, 4].
  - Device main pass: batched indirect-DMA gathers of 528B rows per edge,
    CCE-add fused e_dst gather, leaky-relu+exp on compact logits, per-head
    scaling, and segment reduction on the PE array via on-chip indicator
    matrices (one matmul per 128-edge tile, accumulating in PSUM per
    32-dst window, 4 windows col-tiled per PSUM bank).
  - Epilogue: reciprocal of the denominator column block, scale + ReLU,
    DMA out in slot order; host unpermutes rows to original dst ids.
"""

import math
import numpy as np

from concourse import bacc, bass, mybir, tile
from concourse.bass import IndirectOffsetOnAxis
from concourse.bass_utils import run_bass_kernel_spmd

# ---------------------------------------------------------------- constants
N_CORES = 8
H = 4          # heads
D = 32         # feats per head
HD = H * D     # 128
TC = HD + H    # 132 table columns: [fs | e_src]
W_DST = 32     # dst nodes per window (matmul indicator width)
PGROUP = 4     # windows per PSUM tile (4*32 = 128 partitions)
CHUNK = 32     # slots (128-edge tiles) per gather instruction
NEG_SLOPE = 0.2
SENT_ESRC = -1.0e30
DT = mybir.dt.float32
NP_DT = np.float32
F32 = mybir.dt.float32
I32 = mybir.dt.int32
DEBUG_DUMPS = False


def _cdiv(a, b):
    return (a + b - 1) // b


# ---------------------------------------------------------------- host prep
def _prep(feat_src, feat_dst, w_src, w_dst, attn, src_idx, dst_idx, n_cores):
    n_src, d_in = feat_src.shape
    n_dst = feat_dst.shape[0]
    assert d_in % 128 == 0
    fch = d_in // 128

    ndc = _cdiv(n_dst, n_cores)                    # dsts per core
    ndc_pad = _cdiv(ndc, PGROUP * W_DST) * PGROUP * W_DST
    nwin = ndc_pad // W_DST
    nsrc_pad = _cdiv(n_src, 128) * 128
    sent_row = nsrc_pad                            # sentinel table row id
    nt_src = nsrc_pad // 128
    nt_dst = ndc_pad // 128

    # ---- edge sort by dst
    perm = np.argsort(dst_idx, kind="stable")
    ds = dst_idx[perm]
    ss = src_idx[perm]

    counts = np.zeros((n_cores, nwin), np.int64)
    per_core = []
    for c in range(n_cores):
        lo, hi = np.searchsorted(ds, [c * ndc, min((c + 1) * ndc, n_dst)])
        d_loc = (ds[lo:hi] - c * ndc).astype(np.int64)
        s_loc = ss[lo:hi].astype(np.int64)
        win = d_loc // W_DST
        counts[c] = np.bincount(win, minlength=nwin)
        per_core.append((d_loc, s_loc, win))

    order = np.argsort(-counts, axis=1, kind="stable")     # [n_cores, nwin]
    sorted_counts = np.take_along_axis(counts, order, axis=1)
    rank_max = sorted_counts.max(axis=0)                   # [nwin]
    t_r = np.maximum(1, _cdiv(rank_max, 128)).astype(np.int64)  # tiles/slotrank
    slot_base = np.concatenate([[0], np.cumsum(t_r)])
    stot = int(slot_base[-1])
    n_chunks = _cdiv(stot, CHUNK)
    stot_pad = n_chunks * CHUNK

    # schedule shared by all cores: slot -> (window rank, tile, ntiles)
    slot_sched = []
    for r in range(nwin):
        for t in range(int(t_r[r])):
            slot_sched.append((r, t, int(t_r[r])))
    assert len(slot_sched) == stot

    # ---- per-core edge slot arrays
    idxs_h, idxd_h, dloc_h = [], [], []
    for c in range(n_cores):
        d_loc, s_loc, win = per_core[c]
        e_src_ids = np.full((stot_pad, 128), sent_row, np.int32)
        e_dst_ids = np.zeros((stot_pad, 128), np.int32)
        e_dloc = np.zeros((stot_pad, 128), NP_DT)
        if len(d_loc):
            rank = np.empty(nwin, np.int64)
            rank[order[c]] = np.arange(nwin)
            win_start = np.concatenate([[0], np.cumsum(counts[c])[:-1]])
            posw = np.arange(len(d_loc)) - win_start[win]
            r_of = rank[win]
            slot = slot_base[r_of] + posw // 128
            lane = posw % 128
            e_src_ids[slot, lane] = s_loc
            e_dst_ids[slot, lane] = d_loc
            e_dloc[slot, lane] = (d_loc - win * W_DST).astype(NP_DT)
        # [n_chunks, 128, CHUNK]: arr[i, p, j] = slot i*CHUNK+j, lane p
        def _pack(a):
            return np.ascontiguousarray(
                a.reshape(n_chunks, CHUNK, 128).transpose(0, 2, 1)
            )
        idxs_h.append(_pack(e_src_ids))
        idxd_h.append(_pack(e_dst_ids))
        dloc_h.append(_pack(e_dloc))

    # ---- transposed feature tiles
    def _tiles(feat, npad):
        f = np.zeros((npad, d_in), np.float32)
        f[: feat.shape[0]] = feat
        # [nt, FCH, 128(f), 128(n)]
        return np.ascontiguousarray(
            f.reshape(npad // 128, 128, fch, 128).transpose(0, 2, 3, 1)
        )

    fsT = _tiles(feat_src, nsrc_pad)
    # feat_dst shard rows permuted into slot (sorted-window) order so the
    # e_dst table comes out slot-ordered with compile-time addresses
    fdT = []
    for c in range(n_cores):
        n_here = min(ndc, n_dst - c * ndc)
        fd_slot = np.zeros((ndc_pad, d_in), np.float32)
        for r in range(nwin):
            w = int(order[c][r])
            d0 = w * W_DST
            n = min(W_DST, n_here - d0)
            if n > 0:
                fd_slot[r * W_DST : r * W_DST + n] = \
                    feat_dst[c * ndc + d0 : c * ndc + d0 + n]
        fdT.append(_tiles(fd_slot, ndc_pad))

    # ---- attention selector matrices (pure relayout of attn input)
    a_src = np.zeros((HD, H), np.float32)
    a_dst = np.zeros((HD, H), np.float32)
    for h in range(H):
        a_dst[h * D : (h + 1) * D, h] = attn[h, :D]
        a_src[h * D : (h + 1) * D, h] = attn[h, D:]

    cfg = dict(
        n_src=n_src, n_dst=n_dst, d_in=d_in, fch=fch, ndc=ndc,
        ndc_pad=ndc_pad, nwin=nwin, nsrc_pad=nsrc_pad, sent_row=sent_row,
        nt_src=nt_src, nt_dst=nt_dst, stot=stot, stot_pad=stot_pad,
        n_chunks=n_chunks, slot_sched=slot_sched, n_cores=n_cores,
    )
    common = dict(
        wsrc=np.ascontiguousarray(w_src, np.float32),
        wsrcT=np.ascontiguousarray(w_src.T, np.float32),
        wdstT=np.ascontiguousarray(w_dst.T, np.float32),
        asrc=np.ascontiguousarray(a_src),
        adst=np.ascontiguousarray(a_dst),
        fsT=fsT,
    )
    in_maps = []
    for c in range(n_cores):
        m = dict(common)
        m["fdT"] = fdT[c]
        m["idxs"] = idxs_h[c]
        m["dloc"] = dloc_h[c]
        in_maps.append(m)
    return cfg, in_maps, order


# ---------------------------------------------------------------- device kernel
def _build(nc, tc, cfg):
    fch = cfg["fch"]
    d_in = cfg["d_in"]
    nwin = cfg["nwin"]

    # I/O
    fsT = nc.dram_tensor("fsT", [cfg["nt_src"], fch, 128, 128], F32,
                         kind="ExternalInput")
    fdT = nc.dram_tensor("fdT", [cfg["nt_dst"], fch, 128, 128], F32,
                         kind="ExternalInput")
    wsrc = nc.dram_tensor("wsrc", [d_in, HD], F32, kind="ExternalInput")
    wsrcT = nc.dram_tensor("wsrcT", [HD, d_in], F32, kind="ExternalInput")
    wdstT = nc.dram_tensor("wdstT", [HD, d_in], F32, kind="ExternalInput")
    asrc = nc.dram_tensor("asrc", [HD, H], F32, kind="ExternalInput")
    adst = nc.dram_tensor("adst", [HD, H], F32, kind="ExternalInput")
    idxs = nc.dram_tensor("idxs", [cfg["n_chunks"], 128, CHUNK], I32,
                          kind="ExternalInput")
    dloc = nc.dram_tensor("dloc", [cfg["n_chunks"], 128, CHUNK], DT,
                          kind="ExternalInput")
    out = nc.dram_tensor("out", [cfg["ndc_pad"], HD], F32,
                         kind="ExternalOutput")

    tabkind = "ExternalOutput" if DEBUG_DUMPS else "Internal"
    tab = nc.dram_tensor("tab", [cfg["nsrc_pad"] + 1, TC], DT, kind=tabkind)
    edt = nc.dram_tensor("edt", [cfg["ndc_pad"], H], DT, kind=tabkind)
    if DEBUG_DUMPS:
        dgt = nc.dram_tensor("dgt", [cfg["n_chunks"], 128, CHUNK * TC], DT,
                             kind="ExternalOutput")
        dsbt = nc.dram_tensor("dsbt", [cfg["n_chunks"], 128, CHUNK * W_DST],
                              DT, kind="ExternalOutput")

    import contextlib
    ctx = contextlib.ExitStack()
    with ctx:
        const = ctx.enter_context(tc.tile_pool(name="const", bufs=1))
        sb = ctx.enter_context(tc.tile_pool(name="sb", bufs=3))
        gp = ctx.enter_context(tc.tile_pool(name="gp", bufs=4))
        pp = ctx.enter_context(tc.tile_pool(name="pp", bufs=3, space="PSUM"))

        # ---------------- setup: W_ext = [w_src | M_src], M_dst
        wsT_sb = const.tile([128, d_in], F32, tag="wsT")
        wdT_sb = const.tile([128, d_in], F32, tag="wdT")
        asrc_sb = const.tile([128, H], F32, tag="asrc")
        adst_sb = const.tile([128, H], F32, tag="adst")
        nc.sync.dma_start(out=wsT_sb[:], in_=wsrcT[:, :])
        nc.sync.dma_start(out=wdT_sb[:], in_=wdstT[:, :])
        nc.sync.dma_start(out=asrc_sb[:], in_=asrc[:, :])
        nc.sync.dma_start(out=adst_sb[:], in_=adst[:, :])

        wext = []
        mdst = []
        for k in range(fch):
            we = const.tile([128, TC], F32, tag=f"wext{k}")
            nc.sync.dma_start(out=we[:, :HD],
                              in_=wsrc[k * 128 : (k + 1) * 128, :])
            pm = pp.tile([128, 512], F32, tag="acc")
            nc.tensor.matmul(pm[:, :H], wsT_sb[:, k * 128 : (k + 1) * 128],
                             asrc_sb[:], start=True, stop=True)
            nc.vector.tensor_copy(we[:, HD:TC], pm[:, :H])
            wext.append(we)

            md = const.tile([128, H], F32, tag=f"mdst{k}")
            pm2 = pp.tile([128, 512], F32, tag="acc")
            nc.tensor.matmul(pm2[:, :H], wdT_sb[:, k * 128 : (k + 1) * 128],
                             adst_sb[:], start=True, stop=True)
            nc.vector.tensor_copy(md[:], pm2[:, :H])
            mdst.append(md)

        iota_i = const.tile([128, W_DST], I32, tag="iota_i")
        iota_f = const.tile([128, W_DST], DT, tag="iota_f")
        nc.gpsimd.iota(iota_i[:], pattern=[[1, W_DST]], base=0,
                       channel_multiplier=0)
        nc.vector.tensor_copy(iota_f[:], iota_i[:])

        # ---------------- phase B: e_dst table
        for j in range(cfg["nt_dst"]):
            lh = sb.tile([128, fch * 128], F32, tag="lh")
            nc.sync.dma_start(out=lh[:].rearrange("p (k n) -> p k n", n=128),
                              in_=fdT[j].rearrange("k p n -> p k n"))
            pb = pp.tile([128, 512], F32, tag="acc")
            for k in range(fch):
                nc.tensor.matmul(pb[:, :H], lh[:, k * 128 : (k + 1) * 128],
                                 mdst[k][:], start=(k == 0),
                                 stop=(k == fch - 1))
            eb = sb.tile([128, H], DT, tag="eb")
            nc.vector.tensor_copy(eb[:], pb[:, :H])
            nc.sync.dma_start(out=edt[j * 128 : (j + 1) * 128, :], in_=eb[:])

        # ---------------- phase C: gather table [fs | e_src]
        for j in range(cfg["nt_src"]):
            lh = sb.tile([128, fch * 128], F32, tag="lh")
            nc.sync.dma_start(out=lh[:].rearrange("p (k n) -> p k n", n=128),
                              in_=fsT[j].rearrange("k p n -> p k n"))
            pc = pp.tile([128, 512], F32, tag="acc")
            for k in range(fch):
                nc.tensor.matmul(pc[:, :TC], lh[:, k * 128 : (k + 1) * 128],
                                 wext[k][:], start=(k == 0),
                                 stop=(k == fch - 1))
            tout = sb.tile([128, TC], DT, tag="tout")
            nc.vector.tensor_copy(tout[:], pc[:, :TC])
            nc.sync.dma_start(out=tab[j * 128 : (j + 1) * 128, :], in_=tout[:])

        # sentinel row: fs = 0, e_src = SENT_ESRC
        st = const.tile([1, TC], DT, tag="sent")
        nc.vector.memset(st[:, :HD], 0.0)
        nc.vector.memset(st[:, HD:TC], SENT_ESRC)
        nc.sync.dma_start(out=tab[cfg["nsrc_pad"] : cfg["nsrc_pad"] + 1, :],
                          in_=st[:])

        tc.strict_bb_all_engine_barrier()

        # ---------------- main pass
        sched = cfg["slot_sched"]
        psg = {}
        ewin = {}
        from concourse.masks import make_identity
        ident = const.tile([128, 128], F32, tag="ident")
        make_identity(nc, ident[:])
        for i in range(cfg["n_chunks"]):
            gt = gp.tile([128, CHUNK * TC], DT, tag="gt")
            ixs = sb.tile([128, CHUNK], I32, tag="ixs")
            dl = sb.tile([128, CHUNK], DT, tag="dl")
            nc.sync.dma_start(out=ixs[:], in_=idxs[i])
            nc.sync.dma_start(out=dl[:], in_=dloc[i])

            gt3 = gt[:].rearrange("p (s c) -> p s c", c=TC)
            # HW indirect DMA consumes one offset per output partition, so
            # gather one 128-edge slot per instruction.
            for sl in range(CHUNK):
                nc.gpsimd.indirect_dma_start(
                    out=gt[:, sl * TC : (sl + 1) * TC], out_offset=None,
                    in_=tab[:, :],
                    in_offset=IndirectOffsetOnAxis(ap=ixs[:, sl : sl + 1],
                                                   axis=0))


            # indicator S: [128, CHUNK * W_DST]
            sbt = sb.tile([128, CHUNK * W_DST], DT, tag="sbt")
            nc.vector.tensor_tensor(
                out=sbt[:].rearrange("p (s w) -> p s w", w=W_DST),
                in0=iota_f[:].rearrange("p (o w) -> p o w", o=1).to_broadcast(
                    [128, CHUNK, W_DST]),
                in1=dl[:].rearrange("p (s o) -> p s o", o=1).to_broadcast(
                    [128, CHUNK, W_DST]),
                op=mybir.AluOpType.is_equal)

            # e_dst broadcast onto logit columns: S^T (PE transpose) @
            # edt window slice, added to the gathered e_src columns
            for sl in range(CHUNK):
                s = i * CHUNK + sl
                if s >= cfg["stot"]:
                    break
                r, t, tr = sched[s]
                if t == 0:
                    ew = sb.tile([W_DST, H], DT, tag="ewin", name=f"ew{r}")
                    nc.sync.dma_start(
                        out=ew[:], in_=edt[r * W_DST:(r + 1) * W_DST, :])
                    ewin[r] = ew
                ptr = pp.tile([W_DST, 128], F32, tag="tr", name=f"ptr{s}",
                              bufs=2)
                nc.tensor.transpose(ptr[:], sbt[:, sl * W_DST:(sl + 1) * W_DST],
                                    ident[:])
                stx = sb.tile([W_DST, 128], DT, tag="stx")
                nc.vector.tensor_copy(stx[:], ptr[:])
                peb = pp.tile([128, H], F32, tag="ebp", name=f"peb{s}", bufs=2)
                nc.tensor.matmul(peb[:], stx[:], ewin[r][:],
                                 start=True, stop=True)
                nc.vector.tensor_tensor(out=gt3[:, sl, HD:TC],
                                        in0=gt3[:, sl, HD:TC],
                                        in1=peb[:], op=mybir.AluOpType.add)
                if t == tr - 1:
                    ewin.pop(r, None)

            # leaky relu + exp on logit columns (in place)
            ev = gt3[:, :, HD:TC]
            tmp = sb.tile([128, CHUNK * H], DT, tag="tmp")
            nc.vector.tensor_scalar(out=tmp[:], in0=ev, scalar1=NEG_SLOPE,
                                    scalar2=None, op0=mybir.AluOpType.mult)
            nc.vector.tensor_tensor(out=ev, in0=ev,
                                    in1=tmp[:].rearrange(
                                        "p (s h) -> p s h", h=H),
                                    op=mybir.AluOpType.max)
            nc.scalar.activation(ev, ev, mybir.ActivationFunctionType.Exp)

            # scale fs columns by per-head ex
            for h in range(H):
                fv = gt3[:, :, h * D : (h + 1) * D]
                xv = gt3[:, :, HD + h : HD + h + 1].to_broadcast(
                    [128, CHUNK, D])
                nc.vector.tensor_tensor(out=fv, in0=fv, in1=xv,
                                        op=mybir.AluOpType.mult)

            if DEBUG_DUMPS:
                nc.sync.dma_start(out=dgt[i], in_=gt[:])
                nc.sync.dma_start(out=dsbt[i], in_=sbt[:])

            # segment matmuls
            for sl in range(CHUNK):
                s = i * CHUNK + sl
                if s >= cfg["stot"]:
                    break
                r, t, tr = sched[s]
                g, q = r // PGROUP, r % PGROUP
                if q == 0 and t == 0:
                    psg[g] = pp.tile([128, 512], F32, tag="acc",
                                     name=f"psg{g}")
                nc.tensor.matmul(
                    psg[g][q * W_DST : (q + 1) * W_DST, :TC],
                    sbt[:, sl * W_DST : (sl + 1) * W_DST],
                    gt[:, sl * TC : (sl + 1) * TC],
                    start=(t == 0), stop=(t == tr - 1),
                    tile_position=(0, q * W_DST))
                if q == PGROUP - 1 and t == tr - 1:
                    # epilogue for group g
                    pt = psg.pop(g)
                    dmx = sb.tile([128, H], F32, tag="dmx")
                    rcp = sb.tile([128, H], F32, tag="rcp")
                    nc.vector.tensor_scalar(out=dmx[:], in0=pt[:, HD:TC],
                                            scalar1=1e-30, scalar2=None,
                                            op0=mybir.AluOpType.max)
                    nc.vector.reciprocal(rcp[:], dmx[:])
                    ot = sb.tile([128, HD], F32, tag="ot")
                    for h in range(H):
                        nc.vector.tensor_scalar(
                            out=ot[:, h * D : (h + 1) * D],
                            in0=pt[:, h * D : (h + 1) * D],
                            scalar1=rcp[:, h : h + 1], scalar2=0.0,
                            op0=mybir.AluOpType.mult,
                            op1=mybir.AluOpType.max)
                    nc.sync.dma_start(
                        out=out[g * 128 : (g + 1) * 128, :], in_=ot[:])
    return out


# ---------------------------------------------------------------- entry point
def kernel(feat_src, feat_dst, w_src, w_dst, attn, src_idx, dst_idx,
           _n_cores=N_CORES, _backend="hw", _results_hook=None,
           _runner=None):
    feat_src = np.asarray(feat_src, np.float32)
    feat_dst = np.asarray(feat_dst, np.float32)
    w_src = np.asarray(w_src, np.float32)
    w_dst = np.asarray(w_dst, np.float32)
    attn = np.asarray(attn, np.float32)
    src_idx = np.asarray(src_idx).astype(np.int32)
    dst_idx = np.asarray(dst_idx).astype(np.int32)

    cfg, in_maps, order = _prep(feat_src, feat_dst, w_src, w_dst, attn,
                                src_idx, dst_idx, _n_cores)

    nc = bacc.Bacc("TRN2", target_bir_lowering=False, debug=False)
    with tile.TileContext(nc) as tc:
        _build(nc, tc, cfg)
    nc.compile()

    if _backend == "sim":
        from concourse.bass_interp import CoreSim
        results = []
        for c in range(_n_cores):
            sim = CoreSim(nc, trace=False)
            for name, arr in in_maps[c].items():
                sim.tensor(name)[:] = arr
            sim.simulate(check_with_hw=False)
            results.append({"out": np.array(sim.tensor("out"))})
        res_obj = None
    elif _runner is not None:
        results = _runner(nc, in_maps)
        res_obj = None
    else:
        res_obj = run_bass_kernel_spmd(nc, in_maps,
                                       core_ids=list(range(_n_cores)))
        results = res_obj.results
    if _results_hook is not None:
        _results_hook(res_obj)

    # unpermute slot-ordered outputs back to dst ids
    n_dst = cfg["n_dst"]
    ndc = cfg["ndc"]
    out_full = np.zeros((n_dst, HD), np.float32)
    for c in range(_n_cores):
        oc = results[c]["out"].reshape(cfg["nwin"], W_DST, HD)
        n_here = min(ndc, n_dst - c * ndc)
        for r in range(cfg["nwin"]):
            w = int(order[c][r])
            d0 = w * W_DST
            n = min(W_DST, n_here - d0)
            if n > 0:
                out_full[c * ndc + d0 : c * ndc + d0 + n] = oc[r, :n]
    return out_full



# revision 3
# speedup vs baseline: 1.0450x; 1.0450x over previous
"""GAT-style message passing (nn_MicroConv) on 8 Trainium2 NeuronCores, v2.

Strategy (dst-node partition + native SWDGE row gather):
  - Each core owns N_DST/8 destination nodes; dsts are grouped into
    128-wide PSUM groups, groups sorted by per-core edge count so all
    cores share one compiled schedule (slot counts maxed across cores).
  - Phase 1 (replicated): node transform producing a gather table of
    512B rows [fs bf16 x128 | e_src f32 x4 | pad], split into 4 banks
    of <=25001 rows so indices fit dma_gather's int16; plus a bf16
    e_dst table for the local dst shard.
  - Main pass: per chunk (<=5 groups, <=104 slots of 128 edges), four
    dma_gather instructions (one per bank) pull 512B rows at full DMA
    bandwidth; indicator S built by iota/is_equal; per-slot PE
    transpose of S gives S^T for the e_dst broadcast matmul; chunk-wide
    DVE/ACT ops compute a = exp(lrelu(e_src+e_dst)), scale fs rows and
    write a into the pad columns; one 132-col bf16 matmul per slot
    accumulates [numerator | denominator] per group in PSUM.
  - Epilogue per group: reciprocal, scale, ReLU, DMA out in group-rank
    order; host unpermutes rows to original dst ids.
"""

import numpy as np
import ml_dtypes

from concourse import bacc, bass, mybir, tile
from concourse.bass_utils import run_bass_kernel_spmd
from concourse.masks import make_identity

# ---------------------------------------------------------------- constants
N_CORES = 8
H = 4            # heads
D = 32           # feats per head
HD = H * D       # 128
NBANKS = 4
GW = 128         # dsts per PSUM group
ROWE = 256       # bf16 elements per table row (512 B)
ESOFF = 128      # bf16 col offset of the f32 e_src block
GCAP = 5         # max groups per chunk
SCAP = 104       # max slots per chunk
NEG_SLOPE = 0.2
SENT_ESRC = -1.0e30

F32 = mybir.dt.float32
BF16 = mybir.dt.bfloat16
I16 = mybir.dt.int16
I32 = mybir.dt.int32
BF = ml_dtypes.bfloat16


def _cdiv(a, b):
    return (a + b - 1) // b


# ---------------------------------------------------------------- host prep
def _prep(feat_src, feat_dst, w_src, w_dst, attn, src_idx, dst_idx, n_cores):
    n_src, d_in = feat_src.shape
    n_dst = feat_dst.shape[0]
    fch = d_in // 128
    assert d_in % 128 == 0

    bankw = _cdiv(n_src, NBANKS)
    assert bankw + 1 <= 32768
    ndc = _cdiv(n_dst, n_cores)
    groups = _cdiv(ndc, GW)
    ndcp = groups * GW
    nsrc_pad = _cdiv(n_src, 128) * 128
    nt_src = nsrc_pad // 128
    nt_dst = ndcp // 128

    # ---- edge sort by dst, bucket per core
    perm = np.argsort(dst_idx, kind="stable")
    ds = dst_idx[perm]
    ss = src_idx[perm]

    cnt = np.zeros((n_cores, groups, NBANKS), np.int64)
    per_core = []
    for c in range(n_cores):
        lo, hi = np.searchsorted(ds, [c * ndc, min((c + 1) * ndc, n_dst)])
        d_loc = (ds[lo:hi] - c * ndc).astype(np.int64)
        s = ss[lo:hi].astype(np.int64)
        g = d_loc // GW
        dl = d_loc % GW
        b = s // bankw
        i16 = (s - b * bankw).astype(np.int16)
        cnt[c] = np.bincount(g * NBANKS + b, minlength=groups * NBANKS).reshape(
            groups, NBANKS)
        per_core.append((g, dl, b, i16))

    tot = cnt.sum(axis=2)
    order = np.argsort(-tot, axis=1, kind="stable")        # [c, rank] -> group
    cnt_ranked = np.take_along_axis(cnt, order[:, :, None], axis=1)
    slots_rb = _cdiv(cnt_ranked.max(axis=0), 128)           # [rank, bank]

    # ---- chunks of ranks
    chunks = []
    cur, cs = [], 0
    for r in range(groups):
        s_r = int(slots_rb[r].sum())
        if cur and (len(cur) >= GCAP or cs + s_r > SCAP):
            chunks.append(cur)
            cur, cs = [], 0
        cur.append(r)
        cs += s_r
    if cur:
        chunks.append(cur)

    # ---- schedule: per chunk, bank-major slot list
    chunk_meta = []
    slot_base = np.full((groups, NBANKS), -1, np.int64)
    gslot = 0
    for ch in chunks:
        entries = []
        for b in range(NBANKS):
            for r in ch:
                nb = int(slots_rb[r, b])
                if nb:
                    slot_base[r, b] = gslot + len(entries)
                    entries.extend((r, b) for _ in range(nb))
        first, last = {}, {}
        for i, (r, b) in enumerate(entries):
            first.setdefault(r, i)
            last[r] = i
        # per-bank slot spans within chunk
        bspan = []
        off = 0
        for b in range(NBANKS):
            nb = sum(int(slots_rb[r, b]) for r in ch)
            bspan.append((off, nb))
            off += nb
        chunk_meta.append(dict(ranks=list(ch), entries=entries, first=first,
                               last=last, base=gslot, nsl=len(entries),
                               bspan=bspan))
        gslot += len(entries)
    stot = gslot
    cap = max(m["nsl"] for m in chunk_meta)

    # ---- per-core lane arrays
    ix_h, dl_h = [], []
    for c in range(n_cores):
        g, dl, b, i16 = per_core[c]
        rank_of = np.empty(groups, np.int64)
        rank_of[order[c]] = np.arange(groups)
        r = rank_of[g]
        key = r * NBANKS + b
        o2 = np.argsort(key, kind="stable")
        r2, b2, dl2, i162 = r[o2], b[o2], dl[o2], i16[o2]
        sizes = cnt[c][order[c]].reshape(-1)                # [rank*NBANKS]
        starts = np.concatenate([[0], np.cumsum(sizes)[:-1]])
        posw = np.arange(len(o2)) - starts[r2 * NBANKS + b2]
        slot = slot_base[r2, b2] + posw // 128
        lane = posw % 128

        dlf = np.zeros((128, stot), np.float32)
        dlf[lane, slot] = dl2
        ixu = np.full((16, stot * 8), bankw, np.int16)      # sentinel default
        ixu[lane % 16, slot * 8 + lane // 16] = i162
        ix_h.append(np.ascontiguousarray(np.tile(ixu, (8, 1))))
        dl_h.append(np.ascontiguousarray(dlf.astype(BF)))

    # ---- transposed feature tiles (bf16)
    def _tiles(feat, npad):
        f = np.zeros((npad, d_in), np.float32)
        f[: feat.shape[0]] = feat
        return np.ascontiguousarray(
            f.reshape(npad // 128, 128, fch, 128).transpose(0, 2, 3, 1)
        ).astype(BF)

    fsT = _tiles(feat_src, nsrc_pad)

    fdT = []
    for c in range(n_cores):
        n_here = min(ndc, n_dst - c * ndc)
        fd_slot = np.zeros((ndcp, d_in), np.float32)
        src_rows = order[c][:, None] * GW + np.arange(GW)[None, :]   # [rank, j]
        valid = src_rows < n_here
        rows = np.where(valid, src_rows, 0)
        fd_slot[:] = feat_dst[c * ndc + rows.reshape(-1)] * \
            valid.reshape(-1, 1)
        fdT.append(_tiles(fd_slot, ndcp))

    # ---- attention selector matrices
    a_src = np.zeros((HD, H), np.float32)
    a_dst = np.zeros((HD, H), np.float32)
    for h in range(H):
        a_dst[h * D: (h + 1) * D, h] = attn[h, :D]
        a_src[h * D: (h + 1) * D, h] = attn[h, D:]

    # ---- table store split plan per src tile
    store_plan = []
    for t in range(nt_src):
        lo = t * 128
        hi = min(lo + 128, n_src)
        spans = []
        while lo < hi:
            b = lo // bankw
            run = min(hi, (b + 1) * bankw) - lo
            spans.append((lo - t * 128, run, b, lo - b * bankw))
            lo += run
        store_plan.append(spans)

    cfg = dict(n_src=n_src, n_dst=n_dst, d_in=d_in, fch=fch, bankw=bankw,
               ndc=ndc, groups=groups, ndcp=ndcp, nt_src=nt_src,
               nt_dst=nt_dst, stot=stot, cap=cap, chunk_meta=chunk_meta,
               store_plan=store_plan, n_cores=n_cores)
    common = dict(
        fsT=fsT,
        wsrc=np.ascontiguousarray(w_src.astype(BF)),
        wsrcT=np.ascontiguousarray(w_src.T.astype(np.float32)),
        wdstT=np.ascontiguousarray(w_dst.T.astype(np.float32)),
        asrc=np.ascontiguousarray(a_src),
        adst=np.ascontiguousarray(a_dst),
    )
    in_maps = []
    for c in range(n_cores):
        m = dict(common)
        m["fdT"] = fdT[c]
        m["ixf"] = ix_h[c]
        m["dlf"] = dl_h[c]
        in_maps.append(m)
    return cfg, in_maps, order


# ---------------------------------------------------------------- device kernel
def _build(nc, tc, cfg):
    fch = cfg["fch"]
    d_in = cfg["d_in"]
    bankw = cfg["bankw"]
    stot = cfg["stot"]
    cap = cfg["cap"]

    fsT = nc.dram_tensor("fsT", [cfg["nt_src"], fch, 128, 128], BF16,
                         kind="ExternalInput")
    fdT = nc.dram_tensor("fdT", [cfg["nt_dst"], fch, 128, 128], BF16,
                         kind="ExternalInput")
    wsrc = nc.dram_tensor("wsrc", [d_in, HD], BF16, kind="ExternalInput")
    wsrcT = nc.dram_tensor("wsrcT", [HD, d_in], F32, kind="ExternalInput")
    wdstT = nc.dram_tensor("wdstT", [HD, d_in], F32, kind="ExternalInput")
    asrc = nc.dram_tensor("asrc", [HD, H], F32, kind="ExternalInput")
    adst = nc.dram_tensor("adst", [HD, H], F32, kind="ExternalInput")
    ixf = nc.dram_tensor("ixf", [128, stot * 8], I16, kind="ExternalInput")
    dlf = nc.dram_tensor("dlf", [128, stot], BF16, kind="ExternalInput")
    out = nc.dram_tensor("out", [cfg["ndcp"], HD], F32, kind="ExternalOutput")

    tab = nc.dram_tensor("tab", [NBANKS, bankw + 1, ROWE], BF16,
                         kind="Internal")
    edt = nc.dram_tensor("edt", [cfg["ndcp"], H], BF16, kind="Internal")

    import contextlib
    ctx = contextlib.ExitStack()
    with ctx:
        const = ctx.enter_context(tc.tile_pool(name="const", bufs=1))
        p1ctx = ctx.enter_context(contextlib.ExitStack())
        sb = p1ctx.enter_context(tc.tile_pool(name="sb1", bufs=2))
        pp = p1ctx.enter_context(tc.tile_pool(name="pp1", bufs=2,
                                              space="PSUM"))

        # ---------------- phase 0: build We (bf16 [fch][128, 132]), Mdst
        wsT_sb = const.tile([128, d_in], F32, tag="wsT")
        wdT_sb = const.tile([128, d_in], F32, tag="wdT")
        asrc_sb = const.tile([128, H], F32, tag="asrc")
        adst_sb = const.tile([128, H], F32, tag="adst")
        nc.sync.dma_start(out=wsT_sb[:], in_=wsrcT[:, :])
        nc.sync.dma_start(out=wdT_sb[:], in_=wdstT[:, :])
        nc.sync.dma_start(out=asrc_sb[:], in_=asrc[:, :])
        nc.sync.dma_start(out=adst_sb[:], in_=adst[:, :])

        wext, mdst = [], []
        for k in range(fch):
            we = const.tile([128, HD + H], BF16, tag=f"wext{k}")
            nc.sync.dma_start(out=we[:, :HD],
                              in_=wsrc[k * 128: (k + 1) * 128, :])
            pm = pp.tile([128, 512], F32, tag="p1")
            nc.tensor.matmul(pm[:, :H], wsT_sb[:, k * 128: (k + 1) * 128],
                             asrc_sb[:], start=True, stop=True)
            nc.vector.tensor_copy(we[:, HD: HD + H], pm[:, :H])
            wext.append(we)

            md = const.tile([128, H], BF16, tag=f"mdst{k}")
            pm2 = pp.tile([128, 512], F32, tag="p1")
            nc.tensor.matmul(pm2[:, :H], wdT_sb[:, k * 128: (k + 1) * 128],
                             adst_sb[:], start=True, stop=True)
            nc.vector.tensor_copy(md[:], pm2[:, :H])
            mdst.append(md)

        iota_i = const.tile([128, 128], I32, tag="iota_i")
        iota_f = const.tile([128, 128], BF16, tag="iota_f")
        nc.gpsimd.iota(iota_i[:], pattern=[[1, 128]], base=0,
                       channel_multiplier=0)
        nc.vector.tensor_copy(iota_f[:], iota_i[:])
        ident = const.tile([128, 128], BF16, tag="ident")
        make_identity(nc, ident[:])

        # ---------------- phase B: e_dst table (bf16)
        for j in range(cfg["nt_dst"]):
            lh = sb.tile([128, fch * 128], BF16, tag="lh")
            nc.sync.dma_start(out=lh[:].rearrange("p (k n) -> p k n", n=128),
                              in_=fdT[j].rearrange("k p n -> p k n"))
            pb = pp.tile([128, 512], F32, tag="p1")
            for k in range(fch):
                nc.tensor.matmul(pb[:, :H], lh[:, k * 128: (k + 1) * 128],
                                 mdst[k][:], start=(k == 0),
                                 stop=(k == fch - 1))
            eb = sb.tile([128, H], BF16, tag="eb")
            nc.vector.tensor_copy(eb[:], pb[:, :H])
            nc.sync.dma_start(out=edt[j * 128: (j + 1) * 128, :], in_=eb[:])

        # ---------------- phase C: gather table rows
        for t in range(cfg["nt_src"]):
            lh = sb.tile([128, fch * 128], BF16, tag="lh")
            nc.sync.dma_start(out=lh[:].rearrange("p (k n) -> p k n", n=128),
                              in_=fsT[t].rearrange("k p n -> p k n"))
            pc = pp.tile([128, 512], F32, tag="p1")
            for k in range(fch):
                nc.tensor.matmul(pc[:, : HD + H],
                                 lh[:, k * 128: (k + 1) * 128],
                                 wext[k][:], start=(k == 0),
                                 stop=(k == fch - 1))
            row = sb.tile([128, ROWE], BF16, tag="row")
            nc.vector.memset(row[:, ESOFF + 2 * H:], 0.0)
            nc.scalar.activation(row[:, :HD], pc[:, :HD],
                                 mybir.ActivationFunctionType.Copy)
            nc.vector.tensor_copy(row[:, ESOFF: ESOFF + 2 * H].bitcast(F32),
                                  pc[:, HD: HD + H])
            for (r0, n, b, br) in cfg["store_plan"][t]:
                nc.sync.dma_start(out=tab[b, br: br + n, :],
                                  in_=row[r0: r0 + n, :])

        # sentinel rows (one per bank)
        st = const.tile([1, ROWE], BF16, tag="sent")
        nc.vector.memset(st[:], 0.0)
        nc.vector.memset(st[:, ESOFF: ESOFF + 2 * H].bitcast(F32), SENT_ESRC)
        for b in range(NBANKS):
            nc.sync.dma_start(out=tab[b, bankw: bankw + 1, :], in_=st[:])

        tc.strict_bb_all_engine_barrier()
        p1ctx.close()

        sb = ctx.enter_context(tc.tile_pool(name="sb", bufs=2))
        gp = ctx.enter_context(tc.tile_pool(name="gp", bufs=2))
        pp = ctx.enter_context(tc.tile_pool(name="pp", bufs=2, space="PSUM"))

        # ---------------- main pass
        for ci, meta in enumerate(cfg["chunk_meta"]):
            nsl = meta["nsl"]
            base = meta["base"]

            gt = gp.tile([128, cap * ROWE], BF16, tag="gt")
            gt3 = gt[:].rearrange("p (s c) -> p s c", c=ROWE)
            ix = sb.tile([128, cap * 8], I16, tag="ix")
            dl = sb.tile([128, cap], BF16, tag="dl")
            nc.sync.dma_start(out=ix[:, : nsl * 8],
                              in_=ixf[:, base * 8: (base + nsl) * 8])
            nc.sync.dma_start(out=dl[:, :nsl],
                              in_=dlf[:, base: base + nsl])

            # ucode limit: <=1024 indices (8 slots) per dma_gather
            for b in range(NBANKS):
                s0, nb = meta["bspan"][b]
                for q0 in range(0, nb, 8):
                    qn = min(8, nb - q0)
                    nc.gpsimd.dma_gather(
                        out_ap=gt3[:, s0 + q0: s0 + q0 + qn, :],
                        in_ap=tab[b],
                        idxs_ap=ix[:, (s0 + q0) * 8: (s0 + q0 + qn) * 8],
                        num_idxs=128 * qn,
                        num_idxs_reg=128 * qn,
                        elem_size=ROWE,
                    )

            # indicator S for all slots: S[p, s, d] = (dl[p, s] == d)
            sbt = sb.tile([128, cap * 128], BF16, tag="sbt")
            nc.vector.tensor_tensor(
                out=sbt[:, : nsl * 128].rearrange("p (s d) -> p s d", d=128),
                in0=dl[:, :nsl].rearrange("p (s o) -> p s o", o=1)
                    .to_broadcast([128, nsl, 128]),
                in1=iota_f[:].rearrange("p (o d) -> p o d", o=1)
                    .to_broadcast([128, nsl, 128]),
                op=mybir.AluOpType.is_equal)

            # e_dst tables for this chunk's groups
            ew = {}
            for r in meta["ranks"]:
                t_ = sb.tile([128, H], BF16, tag="ew", name=f"ew{r}", bufs=8)
                nc.sync.dma_start(out=t_[:], in_=edt[r * GW: (r + 1) * GW, :])
                ew[r] = t_

            # per-slot: S^T via PE transpose, e_dst broadcast matmul
            peb = pp.tile([128, 512], F32, tag="peb", bufs=1)
            for sl in range(nsl):
                r, b = meta["entries"][sl]
                ptr = pp.tile([128, 128], BF16, tag="tr", name=f"tr{base+sl}",
                              bufs=2)
                nc.tensor.transpose(ptr[:], sbt[:, sl * 128: (sl + 1) * 128],
                                    ident[:])
                stx = sb.tile([128, 128], BF16, tag="stx", bufs=3)
                nc.scalar.activation(stx[:], ptr[:],
                                     mybir.ActivationFunctionType.Copy)
                nc.tensor.matmul(peb[:, sl * 4: sl * 4 + 4], stx[:],
                                 ew[r][:], start=True, stop=True)

            # chunk-wide: a = exp(lrelu(e_src + e_dst))
            ee = sb.tile([128, cap * 4], F32, tag="ee")
            nc.vector.tensor_tensor(
                out=ee[:, : nsl * 4].rearrange("p (s h) -> p s h", h=4),
                in0=gt3[:, :nsl, ESOFF: ESOFF + 2 * H].bitcast(F32),
                in1=peb[:, : nsl * 4].rearrange("p (s h) -> p s h", h=4),
                op=mybir.AluOpType.add)
            et = sb.tile([128, cap * 4], F32, tag="et")
            nc.vector.tensor_scalar(out=et[:, : nsl * 4],
                                    in0=ee[:, : nsl * 4],
                                    scalar1=NEG_SLOPE, scalar2=None,
                                    op0=mybir.AluOpType.mult)
            nc.vector.tensor_tensor(out=ee[:, : nsl * 4],
                                    in0=ee[:, : nsl * 4],
                                    in1=et[:, : nsl * 4],
                                    op=mybir.AluOpType.max)
            nc.scalar.activation(ee[:, : nsl * 4], ee[:, : nsl * 4],
                                 mybir.ActivationFunctionType.Exp)
            ab = sb.tile([128, cap * 4], BF16, tag="ab")
            nc.vector.tensor_copy(ab[:, : nsl * 4], ee[:, : nsl * 4])
            # write a into the pad columns 128..132 of each gathered row
            nc.vector.tensor_copy(
                gt3[:, :nsl, HD: HD + H],
                ab[:, : nsl * 4].rearrange("p (s h) -> p s h", h=4))
            # scale fs rows by a (per head)
            for h in range(H):
                nc.vector.tensor_tensor(
                    out=gt3[:, :nsl, h * D: (h + 1) * D],
                    in0=gt3[:, :nsl, h * D: (h + 1) * D],
                    in1=ab[:, : nsl * 4]
                        .rearrange("p (s h) -> p s h", h=4)[:, :, h: h + 1]
                        .to_broadcast([128, nsl, D]),
                    op=mybir.AluOpType.mult)

            # segment matmuls + epilogues
            acc = {}
            for sl in range(nsl):
                r, b = meta["entries"][sl]
                if meta["first"][r] == sl:
                    acc[r] = pp.tile([128, 132], F32, tag="acc",
                                     name=f"acc{r}", bufs=5)
                nc.tensor.matmul(
                    acc[r][:, :132],
                    sbt[:, sl * 128: (sl + 1) * 128],
                    gt[:, sl * ROWE: sl * ROWE + 132],
                    start=(meta["first"][r] == sl),
                    stop=(meta["last"][r] == sl))
                if meta["last"][r] == sl:
                    pt = acc.pop(r)
                    dmx = sb.tile([128, H], F32, tag="dmx")
                    rcp = sb.tile([128, H], F32, tag="rcp")
                    nc.vector.tensor_scalar(out=dmx[:], in0=pt[:, HD: HD + H],
                                            scalar1=1e-30, scalar2=None,
                                            op0=mybir.AluOpType.max)
                    nc.vector.reciprocal(rcp[:], dmx[:])
                    ot = sb.tile([128, HD], F32, tag="ot")
                    for h in range(H):
                        nc.vector.tensor_scalar(
                            out=ot[:, h * D: (h + 1) * D],
                            in0=pt[:, h * D: (h + 1) * D],
                            scalar1=rcp[:, h: h + 1], scalar2=0.0,
                            op0=mybir.AluOpType.mult,
                            op1=mybir.AluOpType.max)
                    nc.sync.dma_start(out=out[r * GW: (r + 1) * GW, :],
                                      in_=ot[:])
    return out


# ---------------------------------------------------------------- entry point
def kernel(feat_src, feat_dst, w_src, w_dst, attn, src_idx, dst_idx,
           _n_cores=N_CORES, _backend="hw", _runner=None):
    feat_src = np.asarray(feat_src, np.float32)
    feat_dst = np.asarray(feat_dst, np.float32)
    w_src = np.asarray(w_src, np.float32)
    w_dst = np.asarray(w_dst, np.float32)
    attn = np.asarray(attn, np.float32)
    src_idx = np.asarray(src_idx).astype(np.int64)
    dst_idx = np.asarray(dst_idx).astype(np.int64)

    cfg, in_maps, order = _prep(feat_src, feat_dst, w_src, w_dst, attn,
                                src_idx, dst_idx, _n_cores)

    nc = bacc.Bacc("TRN2", target_bir_lowering=False, debug=False)
    with tile.TileContext(nc) as tc:
        _build(nc, tc, cfg)
    nc.compile()

    if _backend == "sim":
        from concourse.bass_interp import CoreSim
        results = []
        for c in range(_n_cores):
            sim = CoreSim(nc, trace=False, require_nnan=False,
                          require_finite=False)
            for name, arr in in_maps[c].items():
                sim.tensor(name)[:] = arr
            sim.simulate(check_with_hw=False)
            results.append({"out": np.array(sim.tensor("out"))})
    elif _runner is not None:
        results = _runner(nc, in_maps)
    else:
        res = run_bass_kernel_spmd(nc, in_maps,
                                   core_ids=list(range(_n_cores)))
        results = res.results

    n_dst = cfg["n_dst"]
    ndc = cfg["ndc"]
    out_full = np.zeros((n_dst, HD), np.float32)
    for c in range(_n_cores):
        oc = results[c]["out"].reshape(cfg["groups"], GW, HD)
        n_here = min(ndc, n_dst - c * ndc)
        for r in range(cfg["groups"]):
            g = int(order[c][r])
            d0 = g * GW
            n = min(GW, n_here - d0)
            if n > 0:
                out_full[c * ndc + d0: c * ndc + d0 + n] = oc[r, :n]
    return out_full


# revision 8
# speedup vs baseline: 1.1534x; 1.1036x over previous
"""GAT-style message passing (nn_MicroConv) on 8 Trainium2 NeuronCores, v2.

Strategy (dst-node partition + native SWDGE row gather):
  - Each core owns N_DST/8 destination nodes; dsts are grouped into
    128-wide PSUM groups, groups sorted by per-core edge count so all
    cores share one compiled schedule (slot counts maxed across cores).
  - Phase 1 (replicated): node transform producing a gather table of
    512B rows [fs bf16 x128 | e_src f32 x4 | pad], split into 4 banks
    of <=25001 rows so indices fit dma_gather's int16; plus a bf16
    e_dst table for the local dst shard.
  - Main pass: per chunk (<=5 groups, <=104 slots of 128 edges), four
    dma_gather instructions (one per bank) pull 512B rows at full DMA
    bandwidth; indicator S built by iota/is_equal; per-slot PE
    transpose of S gives S^T for the e_dst broadcast matmul; chunk-wide
    DVE/ACT ops compute a = exp(lrelu(e_src+e_dst)), scale fs rows and
    write a into the pad columns; one 132-col bf16 matmul per slot
    accumulates [numerator | denominator] per group in PSUM.
  - Epilogue per group: reciprocal, scale, ReLU, DMA out in group-rank
    order; host unpermutes rows to original dst ids.
"""

import numpy as np
import ml_dtypes

from concourse import bacc, bass, mybir, tile
from concourse.bass_utils import run_bass_kernel_spmd
from concourse.masks import make_identity

# ---------------------------------------------------------------- constants
N_CORES = 8
H = 4            # heads
D = 32           # feats per head
HD = H * D       # 128
NBANKS = 4
GW = 128         # dsts per PSUM group
ROWE = 256       # bf16 elements per table row (512 B)
ESOFF = 128      # bf16 col offset of the f32 e_src block
GCAP = 2         # max groups per chunk
SCAP = 56        # max slots per chunk
TB = 4           # src tiles per phase-1 batch
NEG_SLOPE = 0.2
SENT_ESRC = -1.0e30

F32 = mybir.dt.float32
BF16 = mybir.dt.bfloat16
I16 = mybir.dt.int16
I32 = mybir.dt.int32
BF = ml_dtypes.bfloat16


def _cdiv(a, b):
    return (a + b - 1) // b


# ---------------------------------------------------------------- host prep
def _prep(feat_src, feat_dst, w_src, w_dst, attn, src_idx, dst_idx, n_cores):
    n_src, d_in = feat_src.shape
    n_dst = feat_dst.shape[0]
    fch = d_in // 128
    assert d_in % 128 == 0

    bankw = _cdiv(n_src, NBANKS)
    assert bankw + 1 <= 32768
    ndc = _cdiv(n_dst, n_cores)
    groups = _cdiv(ndc, GW)
    ndcp = groups * GW
    nsrc_pad = _cdiv(n_src, 128) * 128
    nt_src = nsrc_pad // 128
    nt_dst = ndcp // 128

    # ---- edge sort by dst, bucket per core
    perm = np.argsort(dst_idx, kind="stable")
    ds = dst_idx[perm]
    ss = src_idx[perm]

    cnt = np.zeros((n_cores, groups, NBANKS), np.int64)
    per_core = []
    for c in range(n_cores):
        lo, hi = np.searchsorted(ds, [c * ndc, min((c + 1) * ndc, n_dst)])
        d_loc = (ds[lo:hi] - c * ndc).astype(np.int64)
        s = ss[lo:hi].astype(np.int64)
        g = d_loc // GW
        dl = d_loc % GW
        b = s // bankw
        i16 = (s - b * bankw).astype(np.int16)
        cnt[c] = np.bincount(g * NBANKS + b, minlength=groups * NBANKS).reshape(
            groups, NBANKS)
        per_core.append((g, dl, b, i16))

    tot = cnt.sum(axis=2)
    order = np.argsort(-tot, axis=1, kind="stable")        # [c, rank] -> group
    cnt_ranked = np.take_along_axis(cnt, order[:, :, None], axis=1)
    slots_rb = _cdiv(cnt_ranked.max(axis=0), 128)           # [rank, bank]

    # ---- chunks of ranks
    chunks = []
    cur, cs = [], 0
    for r in range(groups):
        s_r = int(slots_rb[r].sum())
        if cur and (len(cur) >= GCAP or cs + s_r > SCAP):
            chunks.append(cur)
            cur, cs = [], 0
        cur.append(r)
        cs += s_r
    if cur:
        chunks.append(cur)

    # ---- schedule: per chunk, bank-major slot list
    chunk_meta = []
    slot_base = np.full((groups, NBANKS), -1, np.int64)
    gslot = 0
    for ch in chunks:
        entries = []
        for b in range(NBANKS):
            for r in ch:
                nb = int(slots_rb[r, b])
                if nb:
                    slot_base[r, b] = gslot + len(entries)
                    entries.extend((r, b) for _ in range(nb))
        first, last = {}, {}
        for i, (r, b) in enumerate(entries):
            first.setdefault(r, i)
            last[r] = i
        # per-bank slot spans within chunk
        bspan = []
        off = 0
        for b in range(NBANKS):
            nb = sum(int(slots_rb[r, b]) for r in ch)
            bspan.append((off, nb))
            off += nb
        chunk_meta.append(dict(ranks=list(ch), entries=entries, first=first,
                               last=last, base=gslot, nsl=len(entries),
                               bspan=bspan))
        gslot += len(entries)
    stot = gslot
    cap = max(m["nsl"] for m in chunk_meta)

    # ---- per-core lane arrays
    ix_h, dl_h = [], []
    for c in range(n_cores):
        g, dl, b, i16 = per_core[c]
        rank_of = np.empty(groups, np.int64)
        rank_of[order[c]] = np.arange(groups)
        r = rank_of[g]
        key = r * NBANKS + b
        o2 = np.argsort(key, kind="stable")
        r2, b2, dl2, i162 = r[o2], b[o2], dl[o2], i16[o2]
        sizes = cnt[c][order[c]].reshape(-1)                # [rank*NBANKS]
        starts = np.concatenate([[0], np.cumsum(sizes)[:-1]])
        posw = np.arange(len(o2)) - starts[r2 * NBANKS + b2]
        slot = slot_base[r2, b2] + posw // 128
        lane = posw % 128

        dlf = np.zeros((128, stot), np.float32)
        dlf[lane, slot] = dl2
        ixu = np.full((16, stot * 8), bankw, np.int16)      # sentinel default
        ixu[lane % 16, slot * 8 + lane // 16] = i162
        ix_h.append(np.ascontiguousarray(np.tile(ixu, (8, 1))))
        dl_h.append(np.ascontiguousarray(dlf.astype(BF)))

    # ---- transposed feature tiles (bf16)
    def _tiles(feat, npad):
        f = np.zeros((npad, d_in), np.float32)
        f[: feat.shape[0]] = feat
        return np.ascontiguousarray(
            f.reshape(npad // 128, 128, fch, 128).transpose(0, 2, 3, 1)
        ).astype(BF)

    fsT = _tiles(feat_src, nsrc_pad)

    fdT = []
    for c in range(n_cores):
        n_here = min(ndc, n_dst - c * ndc)
        fd_slot = np.zeros((ndcp, d_in), np.float32)
        src_rows = order[c][:, None] * GW + np.arange(GW)[None, :]   # [rank, j]
        valid = src_rows < n_here
        rows = np.where(valid, src_rows, 0)
        fd_slot[:] = feat_dst[c * ndc + rows.reshape(-1)] * \
            valid.reshape(-1, 1)
        fdT.append(_tiles(fd_slot, ndcp))

    # ---- attention selector matrices
    a_src = np.zeros((HD, H), np.float32)
    a_dst = np.zeros((HD, H), np.float32)
    for h in range(H):
        a_dst[h * D: (h + 1) * D, h] = attn[h, :D]
        a_src[h * D: (h + 1) * D, h] = attn[h, D:]

    # ---- table store plan per phase-1 batch of TB tiles
    # entries: ("big", b, bank_row, t_rel, ntiles)  - whole aligned tiles
    #          ("small", b, bank_row, t_rel, r0, n) - partial tile rows
    store_plan = []
    for t0 in range(0, nt_src, TB):
        tb = min(TB, nt_src - t0)
        entries = []
        lo = t0 * 128
        hi = min((t0 + tb) * 128, n_src)
        while lo < hi:
            b = lo // bankw
            run = min(hi, (b + 1) * bankw) - lo
            # split [lo, lo+run) into whole-tile aligned part + partials
            s, e = lo, lo + run
            while s < e:
                t_rel = s // 128 - t0
                r0 = s % 128
                if r0 == 0 and e - s >= 128:
                    nt = (e - s) // 128
                    entries.append(("big", b, s - b * bankw, t_rel, nt))
                    s += nt * 128
                else:
                    n = min(e - s, 128 - r0)
                    entries.append(("small", b, s - b * bankw, t_rel, r0, n))
                    s += n
            lo += run
        store_plan.append((t0, tb, entries))

    cfg = dict(n_src=n_src, n_dst=n_dst, d_in=d_in, fch=fch, bankw=bankw,
               ndc=ndc, groups=groups, ndcp=ndcp, nt_src=nt_src,
               nt_dst=nt_dst, stot=stot, cap=cap, chunk_meta=chunk_meta,
               store_plan=store_plan, n_cores=n_cores)
    common = dict(
        fsT=fsT,
        wsrc=np.ascontiguousarray(w_src.astype(BF)),
        wsrcT=np.ascontiguousarray(w_src.T.astype(np.float32)),
        wdstT=np.ascontiguousarray(w_dst.T.astype(np.float32)),
        asrc=np.ascontiguousarray(a_src),
        adst=np.ascontiguousarray(a_dst),
    )
    in_maps = []
    for c in range(n_cores):
        m = dict(common)
        m["fdT"] = fdT[c]
        m["ixf"] = ix_h[c]
        m["dlf"] = dl_h[c]
        in_maps.append(m)
    return cfg, in_maps, order


# ---------------------------------------------------------------- device kernel
def _build(nc, tc, cfg):
    fch = cfg["fch"]
    d_in = cfg["d_in"]
    bankw = cfg["bankw"]
    stot = cfg["stot"]
    cap = cfg["cap"]

    fsT = nc.dram_tensor("fsT", [cfg["nt_src"], fch, 128, 128], BF16,
                         kind="ExternalInput")
    fdT = nc.dram_tensor("fdT", [cfg["nt_dst"], fch, 128, 128], BF16,
                         kind="ExternalInput")
    wsrc = nc.dram_tensor("wsrc", [d_in, HD], BF16, kind="ExternalInput")
    wsrcT = nc.dram_tensor("wsrcT", [HD, d_in], F32, kind="ExternalInput")
    wdstT = nc.dram_tensor("wdstT", [HD, d_in], F32, kind="ExternalInput")
    asrc = nc.dram_tensor("asrc", [HD, H], F32, kind="ExternalInput")
    adst = nc.dram_tensor("adst", [HD, H], F32, kind="ExternalInput")
    ixf = nc.dram_tensor("ixf", [128, stot * 8], I16, kind="ExternalInput")
    dlf = nc.dram_tensor("dlf", [128, stot], BF16, kind="ExternalInput")
    out = nc.dram_tensor("out", [cfg["ndcp"], HD], F32, kind="ExternalOutput")

    tab = nc.dram_tensor("tab", [NBANKS, bankw + 1, ROWE], BF16,
                         kind="Internal")
    edt = nc.dram_tensor("edt", [cfg["ndcp"], H], BF16, kind="Internal")

    import contextlib
    ctx = contextlib.ExitStack()
    with ctx:
        const = ctx.enter_context(tc.tile_pool(name="const", bufs=1))
        p1ctx = ctx.enter_context(contextlib.ExitStack())
        sb = p1ctx.enter_context(tc.tile_pool(name="sb1", bufs=2))
        pp = p1ctx.enter_context(tc.tile_pool(name="pp1", bufs=2,
                                              space="PSUM"))

        # ---------------- phase 0: build We (bf16 [fch][128, 132]), Mdst
        wsT_sb = const.tile([128, d_in], F32, tag="wsT")
        wdT_sb = const.tile([128, d_in], F32, tag="wdT")
        asrc_sb = const.tile([128, H], F32, tag="asrc")
        adst_sb = const.tile([128, H], F32, tag="adst")
        nc.sync.dma_start(out=wsT_sb[:], in_=wsrcT[:, :])
        nc.sync.dma_start(out=wdT_sb[:], in_=wdstT[:, :])
        nc.sync.dma_start(out=asrc_sb[:], in_=asrc[:, :])
        nc.sync.dma_start(out=adst_sb[:], in_=adst[:, :])

        wext, mdst = [], []
        for k in range(fch):
            we = const.tile([128, HD + H], BF16, tag=f"wext{k}")
            nc.sync.dma_start(out=we[:, :HD],
                              in_=wsrc[k * 128: (k + 1) * 128, :])
            pm = pp.tile([128, 512], F32, tag="p1")
            nc.tensor.matmul(pm[:, :H], wsT_sb[:, k * 128: (k + 1) * 128],
                             asrc_sb[:], start=True, stop=True)
            nc.vector.tensor_copy(we[:, HD: HD + H], pm[:, :H])
            wext.append(we)

            md = const.tile([128, H], BF16, tag=f"mdst{k}")
            pm2 = pp.tile([128, 512], F32, tag="p1")
            nc.tensor.matmul(pm2[:, :H], wdT_sb[:, k * 128: (k + 1) * 128],
                             adst_sb[:], start=True, stop=True)
            nc.vector.tensor_copy(md[:], pm2[:, :H])
            mdst.append(md)

        iota_i = const.tile([128, 128], I32, tag="iota_i")
        iota_f = const.tile([128, 128], BF16, tag="iota_f")
        nc.gpsimd.iota(iota_i[:], pattern=[[1, 128]], base=0,
                       channel_multiplier=0)
        nc.vector.tensor_copy(iota_f[:], iota_i[:])
        ident = const.tile([128, 128], BF16, tag="ident")
        make_identity(nc, ident[:])

        # ---------------- phase B: e_dst table (bf16), batched by TB tiles
        for j0 in range(0, cfg["nt_dst"], TB):
            tb = min(TB, cfg["nt_dst"] - j0)
            lh = sb.tile([128, TB * fch * 128], BF16, tag="lhd")
            nc.sync.dma_start(
                out=lh[:, : tb * fch * 128].rearrange(
                    "p (t k n) -> p t k n", k=fch, n=128),
                in_=fdT[j0: j0 + tb].rearrange("t k p n -> p t k n"))
            eb = sb.tile([128, TB * H], BF16, tag="eb")
            for t in range(tb):
                pb = pp.tile([128, 512], F32, tag="p1")
                for k in range(fch):
                    nc.tensor.matmul(
                        pb[:, :H],
                        lh[:, (t * fch + k) * 128: (t * fch + k + 1) * 128],
                        mdst[k][:], start=(k == 0), stop=(k == fch - 1))
                nc.vector.tensor_copy(eb[:, t * H: (t + 1) * H], pb[:, :H])
            nc.sync.dma_start(
                out=edt[j0 * 128: (j0 + tb) * 128, :].rearrange(
                    "(t p) h -> p t h", p=128),
                in_=eb[:, : tb * H].rearrange("p (t h) -> p t h", h=H))

        # ---------------- phase C: gather table rows, batched by TB tiles
        for (t0, tb, entries) in cfg["store_plan"]:
            lh = sb.tile([128, TB * fch * 128], BF16, tag="lh")
            nc.sync.dma_start(
                out=lh[:, : tb * fch * 128].rearrange(
                    "p (t k n) -> p t k n", k=fch, n=128),
                in_=fsT[t0: t0 + tb].rearrange("t k p n -> p t k n"))
            row = sb.tile([128, TB * ROWE], BF16, tag="row")
            row3 = row[:].rearrange("p (t c) -> p t c", c=ROWE)
            nc.vector.memset(row3[:, :tb, ESOFF + 2 * H:], 0.0)
            for t in range(tb):
                pc = pp.tile([128, 512], F32, tag="p1")
                for k in range(fch):
                    nc.tensor.matmul(
                        pc[:, : HD + H],
                        lh[:, (t * fch + k) * 128: (t * fch + k + 1) * 128],
                        wext[k][:], start=(k == 0), stop=(k == fch - 1))
                nc.scalar.activation(row3[:, t, :HD], pc[:, :HD],
                                     mybir.ActivationFunctionType.Copy)
                nc.vector.tensor_copy(
                    row3[:, t, ESOFF: ESOFF + 2 * H].bitcast(F32),
                    pc[:, HD: HD + H])
            for e in entries:
                if e[0] == "big":
                    _, b, br, t_rel, nt = e
                    nc.sync.dma_start(
                        out=tab[b, br: br + nt * 128, :].rearrange(
                            "(t p) c -> p t c", p=128),
                        in_=row3[:, t_rel: t_rel + nt, :])
                else:
                    _, b, br, t_rel, r0, n = e
                    nc.sync.dma_start(out=tab[b, br: br + n, :],
                                      in_=row3[r0: r0 + n, t_rel, :])

        # sentinel rows (one per bank)
        st = const.tile([1, ROWE], BF16, tag="sent")
        nc.vector.memset(st[:], 0.0)
        nc.vector.memset(st[:, ESOFF: ESOFF + 2 * H].bitcast(F32), SENT_ESRC)
        for b in range(NBANKS):
            nc.sync.dma_start(out=tab[b, bankw: bankw + 1, :], in_=st[:])

        tc.strict_bb_all_engine_barrier()
        p1ctx.close()

        sb = ctx.enter_context(tc.tile_pool(name="sb", bufs=2))
        gp = ctx.enter_context(tc.tile_pool(name="gp", bufs=2))
        pp = ctx.enter_context(tc.tile_pool(name="pp", bufs=2, space="PSUM"))

        # ---------------- main pass (2-stage software pipeline)
        gq = [0]  # round-robin gather queue counter

        def stage_a(meta):
            """Gathers, indicator, e_dst expand, a = exp(lrelu(.)), scale."""
            nsl = meta["nsl"]
            base = meta["base"]

            gt = gp.tile([128, cap * ROWE], BF16, tag="gt", bufs=3)
            gt3 = gt[:].rearrange("p (s c) -> p s c", c=ROWE)
            ix = sb.tile([128, cap * 8], I16, tag="ix")
            dl = sb.tile([128, cap], BF16, tag="dl")
            nc.sync.dma_start(out=ix[:, : nsl * 8],
                              in_=ixf[:, base * 8: (base + nsl) * 8])
            nc.sync.dma_start(out=dl[:, :nsl],
                              in_=dlf[:, base: base + nsl])

            # ucode limit: <=1024 indices (8 slots) per dma_gather
            for b in range(NBANKS):
                s0, nb = meta["bspan"][b]
                for q0 in range(0, nb, 8):
                    qn = min(8, nb - q0)
                    nc.gpsimd.dma_gather(
                        out_ap=gt3[:, s0 + q0: s0 + q0 + qn, :],
                        in_ap=tab[b],
                        idxs_ap=ix[:, (s0 + q0) * 8: (s0 + q0 + qn) * 8],
                        num_idxs=128 * qn,
                        num_idxs_reg=128 * qn,
                        elem_size=ROWE,
                        queue_num=gq[0] % 2,
                    )
                    gq[0] += 1

            # indicator S for all slots: S[p, s, d] = (dl[p, s] == d)
            sbt = sb.tile([128, cap * 128], BF16, tag="sbt", bufs=3)
            nc.vector.tensor_tensor(
                out=sbt[:, : nsl * 128].rearrange("p (s d) -> p s d", d=128),
                in0=dl[:, :nsl].rearrange("p (s o) -> p s o", o=1)
                    .to_broadcast([128, nsl, 128]),
                in1=iota_f[:].rearrange("p (o d) -> p o d", o=1)
                    .to_broadcast([128, nsl, 128]),
                op=mybir.AluOpType.is_equal)

            # e_dst tables for this chunk's groups
            ew = {}
            for r in meta["ranks"]:
                t_ = sb.tile([128, H], BF16, tag="ew", name=f"ew{r}", bufs=4)
                nc.sync.dma_start(out=t_[:], in_=edt[r * GW: (r + 1) * GW, :])
                ew[r] = t_

            # per-slot: S^T via PE transpose, e_dst broadcast matmul
            peb = pp.tile([128, 512], F32, tag="peb", bufs=2)
            for sl in range(nsl):
                r, b = meta["entries"][sl]
                ptr = pp.tile([128, 128], BF16, tag="tr", name=f"tr{base+sl}",
                              bufs=2)
                nc.tensor.transpose(ptr[:], sbt[:, sl * 128: (sl + 1) * 128],
                                    ident[:])
                stx = sb.tile([128, 128], BF16, tag="stx", bufs=3)
                nc.scalar.activation(stx[:], ptr[:],
                                     mybir.ActivationFunctionType.Copy)
                nc.tensor.matmul(peb[:, sl * 4: sl * 4 + 4], stx[:],
                                 ew[r][:], start=True, stop=True)

            # chunk-wide: a = exp(lrelu(e_src + e_dst))
            ee = sb.tile([128, cap * 4], F32, tag="ee")
            nc.vector.tensor_tensor(
                out=ee[:, : nsl * 4].rearrange("p (s h) -> p s h", h=4),
                in0=gt3[:, :nsl, ESOFF: ESOFF + 2 * H].bitcast(F32),
                in1=peb[:, : nsl * 4].rearrange("p (s h) -> p s h", h=4),
                op=mybir.AluOpType.add)
            et = sb.tile([128, cap * 4], F32, tag="et")
            nc.vector.tensor_scalar(out=et[:, : nsl * 4],
                                    in0=ee[:, : nsl * 4],
                                    scalar1=NEG_SLOPE, scalar2=None,
                                    op0=mybir.AluOpType.mult)
            nc.vector.tensor_tensor(out=ee[:, : nsl * 4],
                                    in0=ee[:, : nsl * 4],
                                    in1=et[:, : nsl * 4],
                                    op=mybir.AluOpType.max)
            nc.scalar.activation(ee[:, : nsl * 4], ee[:, : nsl * 4],
                                 mybir.ActivationFunctionType.Exp)
            ab = sb.tile([128, cap * 4], BF16, tag="ab")
            nc.vector.tensor_copy(ab[:, : nsl * 4], ee[:, : nsl * 4])
            # write a into the pad columns 128..132 of each gathered row
            nc.vector.tensor_copy(
                gt3[:, :nsl, HD: HD + H],
                ab[:, : nsl * 4].rearrange("p (s h) -> p s h", h=4))
            # scale fs rows by a (per head)
            for h in range(H):
                nc.vector.tensor_tensor(
                    out=gt3[:, :nsl, h * D: (h + 1) * D],
                    in0=gt3[:, :nsl, h * D: (h + 1) * D],
                    in1=ab[:, : nsl * 4]
                        .rearrange("p (s h) -> p s h", h=4)[:, :, h: h + 1]
                        .to_broadcast([128, nsl, D]),
                    op=mybir.AluOpType.mult)
            meta["gt"] = gt
            meta["sbt"] = sbt

        def stage_b(meta):
            """Segment matmuls + per-group epilogues."""
            nsl = meta["nsl"]
            gt = meta.pop("gt")
            sbt = meta.pop("sbt")
            acc = {}
            for sl in range(nsl):
                r, b = meta["entries"][sl]
                if meta["first"][r] == sl:
                    acc[r] = pp.tile([128, 132], F32, tag="acc",
                                     name=f"acc{r}", bufs=4)
                nc.tensor.matmul(
                    acc[r][:, :132],
                    sbt[:, sl * 128: (sl + 1) * 128],
                    gt[:, sl * ROWE: sl * ROWE + 132],
                    start=(meta["first"][r] == sl),
                    stop=(meta["last"][r] == sl))
                if meta["last"][r] == sl:
                    pt = acc.pop(r)
                    dmx = sb.tile([128, H], F32, tag="dmx")
                    rcp = sb.tile([128, H], F32, tag="rcp")
                    nc.vector.tensor_scalar(out=dmx[:], in0=pt[:, HD: HD + H],
                                            scalar1=1e-30, scalar2=None,
                                            op0=mybir.AluOpType.max)
                    nc.vector.reciprocal(rcp[:], dmx[:])
                    ot = sb.tile([128, HD], F32, tag="ot")
                    for h in range(H):
                        nc.vector.tensor_scalar(
                            out=ot[:, h * D: (h + 1) * D],
                            in0=pt[:, h * D: (h + 1) * D],
                            scalar1=rcp[:, h: h + 1], scalar2=0.0,
                            op0=mybir.AluOpType.mult,
                            op1=mybir.AluOpType.max)
                    nc.sync.dma_start(out=out[r * GW: (r + 1) * GW, :],
                                      in_=ot[:])

        prev = None
        for meta in cfg["chunk_meta"]:
            stage_a(meta)
            if prev is not None:
                stage_b(prev)
            prev = meta
        stage_b(prev)
    return out


# ---------------------------------------------------------------- entry point
def kernel(feat_src, feat_dst, w_src, w_dst, attn, src_idx, dst_idx,
           _n_cores=N_CORES, _backend="hw", _runner=None):
    feat_src = np.asarray(feat_src, np.float32)
    feat_dst = np.asarray(feat_dst, np.float32)
    w_src = np.asarray(w_src, np.float32)
    w_dst = np.asarray(w_dst, np.float32)
    attn = np.asarray(attn, np.float32)
    src_idx = np.asarray(src_idx).astype(np.int64)
    dst_idx = np.asarray(dst_idx).astype(np.int64)

    cfg, in_maps, order = _prep(feat_src, feat_dst, w_src, w_dst, attn,
                                src_idx, dst_idx, _n_cores)

    nc = bacc.Bacc("TRN2", target_bir_lowering=False, debug=False,
                   num_swdge_queues=2)
    with tile.TileContext(nc) as tc:
        _build(nc, tc, cfg)
    nc.compile()

    if _backend == "sim":
        from concourse.bass_interp import CoreSim
        results = []
        for c in range(_n_cores):
            sim = CoreSim(nc, trace=False, require_nnan=False,
                          require_finite=False)
            for name, arr in in_maps[c].items():
                sim.tensor(name)[:] = arr
            sim.simulate(check_with_hw=False)
            results.append({"out": np.array(sim.tensor("out"))})
    elif _runner is not None:
        results = _runner(nc, in_maps)
    else:
        res = run_bass_kernel_spmd(nc, in_maps,
                                   core_ids=list(range(_n_cores)))
        results = res.results

    n_dst = cfg["n_dst"]
    ndc = cfg["ndc"]
    out_full = np.zeros((n_dst, HD), np.float32)
    for c in range(_n_cores):
        oc = results[c]["out"].reshape(cfg["groups"], GW, HD)
        n_here = min(ndc, n_dst - c * ndc)
        for r in range(cfg["groups"]):
            g = int(order[c][r])
            d0 = g * GW
            n = min(GW, n_here - d0)
            if n > 0:
                out_full[c * ndc + d0: c * ndc + d0 + n] = oc[r, :n]
    return out_full


# revision 14
# speedup vs baseline: 1.2006x; 1.0410x over previous
"""GAT-style message passing (nn_MicroConv) on 8 Trainium2 NeuronCores, v2.

Strategy (dst-node partition + native SWDGE row gather):
  - Each core owns N_DST/8 destination nodes; dsts are grouped into
    128-wide PSUM groups, groups sorted by per-core edge count so all
    cores share one compiled schedule (slot counts maxed across cores).
  - Phase 1 (replicated): node transform producing a gather table of
    512B rows [fs bf16 x128 | e_src f32 x4 | pad], split into 4 banks
    of <=25001 rows so indices fit dma_gather's int16; plus a bf16
    e_dst table for the local dst shard.
  - Main pass: per chunk (<=5 groups, <=104 slots of 128 edges), four
    dma_gather instructions (one per bank) pull 512B rows at full DMA
    bandwidth; indicator S built by iota/is_equal; per-slot PE
    transpose of S gives S^T for the e_dst broadcast matmul; chunk-wide
    DVE/ACT ops compute a = exp(lrelu(e_src+e_dst)), scale fs rows and
    write a into the pad columns; one 132-col bf16 matmul per slot
    accumulates [numerator | denominator] per group in PSUM.
  - Epilogue per group: reciprocal, scale, ReLU, DMA out in group-rank
    order; host unpermutes rows to original dst ids.
"""

import numpy as np
import ml_dtypes

from concourse import bacc, bass, mybir, tile
from concourse.bass_utils import run_bass_kernel_spmd
from concourse.masks import make_identity

# ---------------------------------------------------------------- constants
N_CORES = 8
H = 4            # heads
D = 32           # feats per head
HD = H * D       # 128
NBANKS = 4
GW = 128         # dsts per PSUM group
ROWE = 256       # bf16 elements per table row (512 B)
ESOFF = 128      # bf16 col offset of the f32 e_src block
GCAP = 2         # max groups per chunk
SCAP = 56        # max slots per chunk
TB = 8           # src tiles per phase-1 batch
NEG_SLOPE = 0.2
SENT_ESRC = -1.0e30

F32 = mybir.dt.float32
BF16 = mybir.dt.bfloat16
I16 = mybir.dt.int16
I32 = mybir.dt.int32
U16 = mybir.dt.uint16
BF = ml_dtypes.bfloat16


def _cdiv(a, b):
    return (a + b - 1) // b


# ---------------------------------------------------------------- host prep
def _prep(feat_src, feat_dst, w_src, w_dst, attn, src_idx, dst_idx, n_cores):
    n_src, d_in = feat_src.shape
    n_dst = feat_dst.shape[0]
    fch = d_in // 128
    assert d_in % 128 == 0

    bankw = _cdiv(n_src, NBANKS)
    assert bankw + 1 <= 32768
    ndc = _cdiv(n_dst, n_cores)
    groups = _cdiv(ndc, GW)
    ndcp = groups * GW
    nsrc_pad = _cdiv(n_src, 128) * 128
    nt_src = nsrc_pad // 128
    nt_dst = ndcp // 128

    # ---- edge sort by dst, bucket per core
    perm = np.argsort(dst_idx, kind="stable")
    ds = dst_idx[perm]
    ss = src_idx[perm]

    cnt = np.zeros((n_cores, groups, NBANKS), np.int64)
    per_core = []
    for c in range(n_cores):
        lo, hi = np.searchsorted(ds, [c * ndc, min((c + 1) * ndc, n_dst)])
        d_loc = (ds[lo:hi] - c * ndc).astype(np.int64)
        s = ss[lo:hi].astype(np.int64)
        g = d_loc // GW
        dl = d_loc % GW
        b = s // bankw
        i16 = (s - b * bankw).astype(np.int16)
        cnt[c] = np.bincount(g * NBANKS + b, minlength=groups * NBANKS).reshape(
            groups, NBANKS)
        per_core.append((g, dl, b, i16))

    tot = cnt.sum(axis=2)
    order = np.argsort(-tot, axis=1, kind="stable")        # [c, rank] -> group
    cnt_ranked = np.take_along_axis(cnt, order[:, :, None], axis=1)
    slots_rb = _cdiv(cnt_ranked.max(axis=0), 128)           # [rank, bank]

    # ---- chunks of ranks
    chunks = []
    cur, cs = [], 0
    for r in range(groups):
        s_r = int(slots_rb[r].sum())
        if cur and (len(cur) >= GCAP or cs + s_r > SCAP):
            chunks.append(cur)
            cur, cs = [], 0
        cur.append(r)
        cs += s_r
    if cur:
        chunks.append(cur)

    # ---- schedule: per chunk, bank-major slot list
    chunk_meta = []
    slot_base = np.full((groups, NBANKS), -1, np.int64)
    gslot = 0
    for ch in chunks:
        entries = []
        for b in range(NBANKS):
            for r in ch:
                nb = int(slots_rb[r, b])
                if nb:
                    slot_base[r, b] = gslot + len(entries)
                    entries.extend((r, b) for _ in range(nb))
        first, last = {}, {}
        for i, (r, b) in enumerate(entries):
            first.setdefault(r, i)
            last[r] = i
        # per-bank slot spans within chunk
        bspan = []
        off = 0
        for b in range(NBANKS):
            nb = sum(int(slots_rb[r, b]) for r in ch)
            bspan.append((off, nb))
            off += nb
        chunk_meta.append(dict(ranks=list(ch), entries=entries, first=first,
                               last=last, base=gslot, nsl=len(entries),
                               bspan=bspan))
        gslot += len(entries)
    stot = gslot
    cap = max(m["nsl"] for m in chunk_meta)

    # ---- per-core lane arrays
    ix_h, dl_h = [], []
    for c in range(n_cores):
        g, dl, b, i16 = per_core[c]
        rank_of = np.empty(groups, np.int64)
        rank_of[order[c]] = np.arange(groups)
        r = rank_of[g]
        key = r * NBANKS + b
        o2 = np.argsort(key, kind="stable")
        r2, b2, dl2, i162 = r[o2], b[o2], dl[o2], i16[o2]
        sizes = cnt[c][order[c]].reshape(-1)                # [rank*NBANKS]
        starts = np.concatenate([[0], np.cumsum(sizes)[:-1]])
        posw = np.arange(len(o2)) - starts[r2 * NBANKS + b2]
        slot = slot_base[r2, b2] + posw // 128
        lane = posw % 128

        dlf = np.zeros((128, stot), np.float32)
        dlf[lane, slot] = dl2
        ixu = np.full((16, stot * 8), bankw, np.int16)      # sentinel default
        ixu[lane % 16, slot * 8 + lane // 16] = i162
        ix_h.append(np.ascontiguousarray(np.tile(ixu, (8, 1))))
        dl_h.append(np.ascontiguousarray(dlf.astype(BF)))


    # ---- transposed feature tiles (bf16)
    def _tiles(feat, npad):
        f = np.zeros((npad, d_in), np.float32)
        f[: feat.shape[0]] = feat
        return np.ascontiguousarray(
            f.reshape(npad // 128, 128, fch, 128).transpose(0, 2, 3, 1)
        ).astype(BF)

    fsT = _tiles(feat_src, nsrc_pad)

    fdT = []
    for c in range(n_cores):
        n_here = min(ndc, n_dst - c * ndc)
        fd_slot = np.zeros((ndcp, d_in), np.float32)
        src_rows = order[c][:, None] * GW + np.arange(GW)[None, :]   # [rank, j]
        valid = src_rows < n_here
        rows = np.where(valid, src_rows, 0)
        fd_slot[:] = feat_dst[c * ndc + rows.reshape(-1)] * \
            valid.reshape(-1, 1)
        fdT.append(_tiles(fd_slot, ndcp))

    # ---- attention selector matrices
    a_src = np.zeros((HD, H), np.float32)
    a_dst = np.zeros((HD, H), np.float32)
    for h in range(H):
        a_dst[h * D: (h + 1) * D, h] = attn[h, :D]
        a_src[h * D: (h + 1) * D, h] = attn[h, D:]

    # ---- table store plan per phase-1 batch of TB tiles
    # entries: ("big", b, bank_row, t_rel, ntiles)  - whole aligned tiles
    #          ("small", b, bank_row, t_rel, r0, n) - partial tile rows
    store_plan = []
    for t0 in range(0, nt_src, TB):
        tb = min(TB, nt_src - t0)
        entries = []
        lo = t0 * 128
        hi = min((t0 + tb) * 128, n_src)
        while lo < hi:
            b = lo // bankw
            run = min(hi, (b + 1) * bankw) - lo
            # split [lo, lo+run) into whole-tile aligned part + partials
            s, e = lo, lo + run
            while s < e:
                t_rel = s // 128 - t0
                r0 = s % 128
                if r0 == 0 and e - s >= 128:
                    nt = (e - s) // 128
                    entries.append(("big", b, s - b * bankw, t_rel, nt))
                    s += nt * 128
                else:
                    n = min(e - s, 128 - r0)
                    entries.append(("small", b, s - b * bankw, t_rel, r0, n))
                    s += n
            lo += run
        store_plan.append((t0, tb, entries))

    cfg = dict(n_src=n_src, n_dst=n_dst, d_in=d_in, fch=fch, bankw=bankw,
               ndc=ndc, groups=groups, ndcp=ndcp, nt_src=nt_src,
               nt_dst=nt_dst, stot=stot, cap=cap, chunk_meta=chunk_meta,
               store_plan=store_plan, n_cores=n_cores)
    common = dict(
        fsT=fsT,
        wsrc=np.ascontiguousarray(w_src.astype(BF)),
        wsrcT=np.ascontiguousarray(w_src.T.astype(np.float32)),
        wdstT=np.ascontiguousarray(w_dst.T.astype(np.float32)),
        asrc=np.ascontiguousarray(a_src),
        adst=np.ascontiguousarray(a_dst),
    )
    in_maps = []
    for c in range(n_cores):
        m = dict(common)
        m["fdT"] = fdT[c]
        m["ixf"] = ix_h[c]
        m["dlf"] = dl_h[c]
        in_maps.append(m)
    return cfg, in_maps, order


# ---------------------------------------------------------------- device kernel
def _build(nc, tc, cfg):
    fch = cfg["fch"]
    d_in = cfg["d_in"]
    bankw = cfg["bankw"]
    stot = cfg["stot"]
    cap = cfg["cap"]

    fsT = nc.dram_tensor("fsT", [cfg["nt_src"], fch, 128, 128], BF16,
                         kind="ExternalInput")
    fdT = nc.dram_tensor("fdT", [cfg["nt_dst"], fch, 128, 128], BF16,
                         kind="ExternalInput")
    wsrc = nc.dram_tensor("wsrc", [d_in, HD], BF16, kind="ExternalInput")
    wsrcT = nc.dram_tensor("wsrcT", [HD, d_in], F32, kind="ExternalInput")
    wdstT = nc.dram_tensor("wdstT", [HD, d_in], F32, kind="ExternalInput")
    asrc = nc.dram_tensor("asrc", [HD, H], F32, kind="ExternalInput")
    adst = nc.dram_tensor("adst", [HD, H], F32, kind="ExternalInput")
    ixf = nc.dram_tensor("ixf", [128, stot * 8], I16, kind="ExternalInput")
    dlf = nc.dram_tensor("dlf", [128, stot], BF16, kind="ExternalInput")
    out = nc.dram_tensor("out", [cfg["ndcp"], HD], F32, kind="ExternalOutput")

    tab = nc.dram_tensor("tab", [NBANKS, bankw + 1, ROWE], BF16,
                         kind="Internal")
    edt = nc.dram_tensor("edt", [cfg["ndcp"], H], BF16, kind="Internal")

    import contextlib
    ctx = contextlib.ExitStack()
    with ctx:
        const = ctx.enter_context(tc.tile_pool(name="const", bufs=1))
        p1ctx = ctx.enter_context(contextlib.ExitStack())
        sb = p1ctx.enter_context(tc.tile_pool(name="sb1", bufs=3))
        pp = p1ctx.enter_context(tc.tile_pool(name="pp1", bufs=2,
                                              space="PSUM"))

        # ---------------- phase 0: build We (bf16 [fch][128, 132]), Mdst
        wsT_sb = const.tile([128, d_in], F32, tag="wsT")
        wdT_sb = const.tile([128, d_in], F32, tag="wdT")
        asrc_sb = const.tile([128, H], F32, tag="asrc")
        adst_sb = const.tile([128, H], F32, tag="adst")
        nc.sync.dma_start(out=wsT_sb[:], in_=wsrcT[:, :])
        nc.sync.dma_start(out=wdT_sb[:], in_=wdstT[:, :])
        nc.sync.dma_start(out=asrc_sb[:], in_=asrc[:, :])
        nc.sync.dma_start(out=adst_sb[:], in_=adst[:, :])

        wext, mdst = [], []
        for k in range(fch):
            we = const.tile([128, HD + H], BF16, tag=f"wext{k}")
            nc.sync.dma_start(out=we[:, :HD],
                              in_=wsrc[k * 128: (k + 1) * 128, :])
            pm = pp.tile([128, 512], F32, tag="p1", bufs=4)
            nc.tensor.matmul(pm[:, :H], wsT_sb[:, k * 128: (k + 1) * 128],
                             asrc_sb[:], start=True, stop=True)
            nc.vector.tensor_copy(we[:, HD: HD + H], pm[:, :H])
            wext.append(we)

            md = const.tile([128, H], BF16, tag=f"mdst{k}")
            pm2 = pp.tile([128, 512], F32, tag="p1", bufs=4)
            nc.tensor.matmul(pm2[:, :H], wdT_sb[:, k * 128: (k + 1) * 128],
                             adst_sb[:], start=True, stop=True)
            nc.vector.tensor_copy(md[:], pm2[:, :H])
            mdst.append(md)

        iota_i = const.tile([128, 128], I32, tag="iota_i")
        iota_f = const.tile([128, 128], BF16, tag="iota_f")
        nc.gpsimd.iota(iota_i[:], pattern=[[1, 128]], base=0,
                       channel_multiplier=0)
        nc.vector.tensor_copy(iota_f[:], iota_i[:])
        ident = const.tile([128, 128], BF16, tag="ident")
        make_identity(nc, ident[:])

        # ---------------- phase B: e_dst table (bf16), batched by TB tiles
        for j0 in range(0, cfg["nt_dst"], TB):
            tb = min(TB, cfg["nt_dst"] - j0)
            lh = sb.tile([128, TB * fch * 128], BF16, tag="lhd")
            nc.sync.dma_start(
                out=lh[:, : tb * fch * 128].rearrange(
                    "p (t k n) -> p t k n", k=fch, n=128),
                in_=fdT[j0: j0 + tb].rearrange("t k p n -> p t k n"))
            eb = sb.tile([128, TB * H], BF16, tag="eb")
            for t in range(tb):
                pb = pp.tile([128, 512], F32, tag="p1", bufs=4)
                for k in range(fch):
                    nc.tensor.matmul(
                        pb[:, :H],
                        lh[:, (t * fch + k) * 128: (t * fch + k + 1) * 128],
                        mdst[k][:], start=(k == 0), stop=(k == fch - 1))
                nc.vector.tensor_copy(eb[:, t * H: (t + 1) * H], pb[:, :H])
            nc.sync.dma_start(
                out=edt[j0 * 128: (j0 + tb) * 128, :].rearrange(
                    "(t p) h -> p t h", p=128),
                in_=eb[:, : tb * H].rearrange("p (t h) -> p t h", h=H))

        # ---------------- phase C: gather table rows, batched by TB tiles
        for (t0, tb, entries) in cfg["store_plan"]:
            lh = sb.tile([128, TB * fch * 128], BF16, tag="lh")
            nc.sync.dma_start(
                out=lh[:, : tb * fch * 128].rearrange(
                    "p (t k n) -> p t k n", k=fch, n=128),
                in_=fsT[t0: t0 + tb].rearrange("t k p n -> p t k n"))
            row = sb.tile([128, TB * ROWE], BF16, tag="row")
            row3 = row[:].rearrange("p (t c) -> p t c", c=ROWE)
            nc.vector.memset(row3[:, :tb, ESOFF + 2 * H:], 0.0)
            for t in range(tb):
                pc = pp.tile([128, 512], F32, tag="p1", bufs=4)
                for k in range(fch):
                    nc.tensor.matmul(
                        pc[:, : HD + H],
                        lh[:, (t * fch + k) * 128: (t * fch + k + 1) * 128],
                        wext[k][:], start=(k == 0), stop=(k == fch - 1))
                if t % 2 == 0:
                    nc.scalar.activation(row3[:, t, :HD], pc[:, :HD],
                                         mybir.ActivationFunctionType.Copy)
                else:
                    nc.vector.tensor_copy(row3[:, t, :HD], pc[:, :HD])
                nc.vector.tensor_copy(
                    row3[:, t, ESOFF: ESOFF + 2 * H].bitcast(F32),
                    pc[:, HD: HD + H])
            for e in entries:
                if e[0] == "big":
                    _, b, br, t_rel, nt = e
                    nc.sync.dma_start(
                        out=tab[b, br: br + nt * 128, :].rearrange(
                            "(t p) c -> p t c", p=128),
                        in_=row3[:, t_rel: t_rel + nt, :])
                else:
                    _, b, br, t_rel, r0, n = e
                    nc.sync.dma_start(out=tab[b, br: br + n, :],
                                      in_=row3[r0: r0 + n, t_rel, :])

        # sentinel rows (one per bank)
        st = const.tile([1, ROWE], BF16, tag="sent")
        nc.vector.memset(st[:], 0.0)
        nc.vector.memset(st[:, ESOFF: ESOFF + 2 * H].bitcast(F32), SENT_ESRC)
        for b in range(NBANKS):
            nc.sync.dma_start(out=tab[b, bankw: bankw + 1, :], in_=st[:])

        tc.strict_bb_all_engine_barrier()
        p1ctx.close()

        sb = ctx.enter_context(tc.tile_pool(name="sb", bufs=2))
        gp = ctx.enter_context(tc.tile_pool(name="gp", bufs=2))
        pp = ctx.enter_context(tc.tile_pool(name="pp", bufs=2, space="PSUM"))

        # ---------------- main pass (2-stage software pipeline)
        gq = [0]  # round-robin gather queue counter

        def stage_g(meta):
            """Index/dloc loads + row gathers (runs 2 chunks ahead)."""
            nsl = meta["nsl"]
            base = meta["base"]

            gt = gp.tile([128, cap * ROWE], BF16, tag="gt", bufs=5)
            gt3 = gt[:].rearrange("p (s c) -> p s c", c=ROWE)
            ix = sb.tile([128, cap * 8], I16, tag="ix", bufs=3)
            dl = sb.tile([128, cap], BF16, tag="dl", bufs=3)
            nc.sync.dma_start(out=ix[:, : nsl * 8],
                              in_=ixf[:, base * 8: (base + nsl) * 8])
            nc.sync.dma_start(out=dl[:, :nsl],
                              in_=dlf[:, base: base + nsl])

            # ucode limit: <=1024 indices (8 slots) per dma_gather
            for b in range(NBANKS):
                s0, nb = meta["bspan"][b]
                for q0 in range(0, nb, 8):
                    qn = min(8, nb - q0)
                    nc.gpsimd.dma_gather(
                        out_ap=gt3[:, s0 + q0: s0 + q0 + qn, :],
                        in_ap=tab[b],
                        idxs_ap=ix[:, (s0 + q0) * 8: (s0 + q0 + qn) * 8],
                        num_idxs=128 * qn,
                        num_idxs_reg=128 * qn,
                        elem_size=ROWE,
                        queue_num=gq[0] % 4,
                    )
                    gq[0] += 1
            meta["gt"] = gt
            meta["dl"] = dl

        def stage_c(meta):
            """Indicator, e_dst expand, a = exp(lrelu(.)), row scaling."""
            nsl = meta["nsl"]
            base = meta["base"]
            gt = meta["gt"]
            dl = meta["dl"]
            gt3 = gt[:].rearrange("p (s c) -> p s c", c=ROWE)

            # indicator S for all slots: S[p, s, d] = (dl[p, s] == d)
            sbt = sb.tile([128, cap * 128], BF16, tag="sbt", bufs=3)
            nc.vector.tensor_tensor(
                out=sbt[:, : nsl * 128].rearrange("p (s d) -> p s d", d=128),
                in0=dl[:, :nsl].rearrange("p (s o) -> p s o", o=1)
                    .to_broadcast([128, nsl, 128]),
                in1=iota_f[:].rearrange("p (o d) -> p o d", o=1)
                    .to_broadcast([128, nsl, 128]),
                op=mybir.AluOpType.is_equal)

            # e_dst tables for this chunk's groups
            ew = {}
            for r in meta["ranks"]:
                t_ = sb.tile([128, H], BF16, tag="ew", name=f"ew{r}", bufs=4)
                nc.sync.dma_start(out=t_[:], in_=edt[r * GW: (r + 1) * GW, :])
                ew[r] = t_

            # per-slot: S^T via PE transpose, e_dst broadcast matmul
            peb = pp.tile([128, 512], F32, tag="peb", bufs=1)
            for sl in range(nsl):
                r, b = meta["entries"][sl]
                ptr = pp.tile([128, 128], BF16, tag="tr", name=f"tr{base+sl}",
                              bufs=3)
                nc.tensor.transpose(ptr[:], sbt[:, sl * 128: (sl + 1) * 128],
                                    ident[:])
                stx = sb.tile([128, 128], BF16, tag="stx", bufs=4)
                nc.scalar.activation(stx[:], ptr[:],
                                     mybir.ActivationFunctionType.Copy)
                nc.tensor.matmul(peb[:, sl * 4: sl * 4 + 4], stx[:],
                                 ew[r][:], start=True, stop=True)

            # chunk-wide: a = exp(lrelu(e_src + e_dst))
            ee = sb.tile([128, cap * 4], F32, tag="ee")
            nc.vector.tensor_tensor(
                out=ee[:, : nsl * 4].rearrange("p (s h) -> p s h", h=4),
                in0=gt3[:, :nsl, ESOFF: ESOFF + 2 * H].bitcast(F32),
                in1=peb[:, : nsl * 4].rearrange("p (s h) -> p s h", h=4),
                op=mybir.AluOpType.add)
            et = sb.tile([128, cap * 4], F32, tag="et")
            nc.vector.tensor_scalar(out=et[:, : nsl * 4],
                                    in0=ee[:, : nsl * 4],
                                    scalar1=NEG_SLOPE, scalar2=None,
                                    op0=mybir.AluOpType.mult)
            nc.vector.tensor_tensor(out=ee[:, : nsl * 4],
                                    in0=ee[:, : nsl * 4],
                                    in1=et[:, : nsl * 4],
                                    op=mybir.AluOpType.max)
            nc.scalar.activation(ee[:, : nsl * 4], ee[:, : nsl * 4],
                                 mybir.ActivationFunctionType.Exp)
            ab = sb.tile([128, cap * 4], BF16, tag="ab")
            nc.vector.tensor_copy(ab[:, : nsl * 4], ee[:, : nsl * 4])
            # write a into the pad columns 128..132 of each gathered row
            nc.vector.tensor_copy(
                gt3[:, :nsl, HD: HD + H],
                ab[:, : nsl * 4].rearrange("p (s h) -> p s h", h=4))
            # scale fs rows by a (per head)
            for h in range(H):
                nc.vector.tensor_tensor(
                    out=gt3[:, :nsl, h * D: (h + 1) * D],
                    in0=gt3[:, :nsl, h * D: (h + 1) * D],
                    in1=ab[:, : nsl * 4]
                        .rearrange("p (s h) -> p s h", h=4)[:, :, h: h + 1]
                        .to_broadcast([128, nsl, D]),
                    op=mybir.AluOpType.mult)
            meta["sbt"] = sbt

        def stage_b(meta):
            """Segment matmuls + per-group epilogues."""
            nsl = meta["nsl"]
            gt = meta.pop("gt")
            sbt = meta.pop("sbt")
            acc = {}
            for sl in range(nsl):
                r, b = meta["entries"][sl]
                if meta["first"][r] == sl:
                    acc[r] = pp.tile([128, 132], F32, tag="acc",
                                     name=f"acc{r}", bufs=4)
                nc.tensor.matmul(
                    acc[r][:, :132],
                    sbt[:, sl * 128: (sl + 1) * 128],
                    gt[:, sl * ROWE: sl * ROWE + 132],
                    start=(meta["first"][r] == sl),
                    stop=(meta["last"][r] == sl))
                if meta["last"][r] == sl:
                    pt = acc.pop(r)
                    dmx = sb.tile([128, H], F32, tag="dmx")
                    rcp = sb.tile([128, H], F32, tag="rcp")
                    nc.vector.tensor_scalar(out=dmx[:], in0=pt[:, HD: HD + H],
                                            scalar1=1e-30, scalar2=None,
                                            op0=mybir.AluOpType.max)
                    nc.vector.reciprocal(rcp[:], dmx[:])
                    ot = sb.tile([128, HD], F32, tag="ot")
                    for h in range(H):
                        nc.vector.tensor_scalar(
                            out=ot[:, h * D: (h + 1) * D],
                            in0=pt[:, h * D: (h + 1) * D],
                            scalar1=rcp[:, h: h + 1], scalar2=0.0,
                            op0=mybir.AluOpType.mult,
                            op1=mybir.AluOpType.max)
                    nc.sync.dma_start(out=out[r * GW: (r + 1) * GW, :],
                                      in_=ot[:])

        metas = cfg["chunk_meta"]
        n = len(metas)
        for k in range(n + 2):
            if k < n:
                stage_g(metas[k])
            if 1 <= k < n + 1:
                stage_c(metas[k - 1])
            if k >= 2:
                stage_b(metas[k - 2])
    return out


# ---------------------------------------------------------------- entry point
def kernel(feat_src, feat_dst, w_src, w_dst, attn, src_idx, dst_idx,
           _n_cores=N_CORES, _backend="hw", _runner=None):
    feat_src = np.asarray(feat_src, np.float32)
    feat_dst = np.asarray(feat_dst, np.float32)
    w_src = np.asarray(w_src, np.float32)
    w_dst = np.asarray(w_dst, np.float32)
    attn = np.asarray(attn, np.float32)
    src_idx = np.asarray(src_idx).astype(np.int64)
    dst_idx = np.asarray(dst_idx).astype(np.int64)

    cfg, in_maps, order = _prep(feat_src, feat_dst, w_src, w_dst, attn,
                                src_idx, dst_idx, _n_cores)

    nc = bacc.Bacc("TRN2", target_bir_lowering=False, debug=False,
                   num_swdge_queues=4)
    with tile.TileContext(nc) as tc:
        _build(nc, tc, cfg)
    nc.compile()

    if _backend == "sim":
        from concourse.bass_interp import CoreSim
        results = []
        for c in range(_n_cores):
            sim = CoreSim(nc, trace=False, require_nnan=False,
                          require_finite=False)
            for name, arr in in_maps[c].items():
                sim.tensor(name)[:] = arr
            sim.simulate(check_with_hw=False)
            results.append({"out": np.array(sim.tensor("out"))})
    elif _runner is not None:
        results = _runner(nc, in_maps)
    else:
        res = run_bass_kernel_spmd(nc, in_maps,
                                   core_ids=list(range(_n_cores)))
        results = res.results

    n_dst = cfg["n_dst"]
    ndc = cfg["ndc"]
    out_full = np.zeros((n_dst, HD), np.float32)
    for c in range(_n_cores):
        oc = results[c]["out"].reshape(cfg["groups"], GW, HD)
        n_here = min(ndc, n_dst - c * ndc)
        for r in range(cfg["groups"]):
            g = int(order[c][r])
            d0 = g * GW
            n = min(GW, n_here - d0)
            if n > 0:
                out_full[c * ndc + d0: c * ndc + d0 + n] = oc[r, :n]
    return out_full
